# revision 4
# baseline (speedup 1.0000x reference)
"""Trainium2 Bass kernel for nn_GeneSetPlaceholderAggregator.

Computes out[b,s,d] = sum_g x[b,g,d] * W[s,g]  (einsum 'bgd,sg->bsd')
with B=64, G=20000, D=16, S=128.

Strategy:
- Shard the contraction axis G across 8 cores (2500 genes each, zero-padded
  to 2560 = 20 chunks of 128).  Each core computes a full partial output
  [S=128, B*D=1024] via PSUM-accumulated matmuls (contraction on the
  partition dim); the host sums the 8 partials.
- fp16 operands: halves HBM traffic vs fp32 and runs the PE at 1 cycle/row
  with fast-weight-load (fp32r measured 1.5 cyc/row with serial LDWEIGHTS).
  Rounding error ~4e-4 rel, far under the 2e-2 gate.
- Warm-up matmuls on a zeroed tile during the first-DMA latency window trip
  the HAM activity ramp so real matmuls run at 2.4 GHz.
- Host packs per-gene rows [x | W] partition-major ([128, chunk, row]) so
  every DMA descriptor is a long contiguous run per partition.
- Output returned as fp16 [S, B*D] (host sums partials in fp32).
"""

import numpy as np

import concourse.mybir as mybir
from concourse import bass
from concourse.bacc import Bacc
from concourse.bass_utils import run_bass_kernel_spmd
from concourse.tile import TileContext

B, G, D, S = 64, 20000, 16, 128
N_CORES = 8
K = 128                        # contraction tile = partition dim
N_CHUNKS = 20                  # chunks per core
G_LOC = K * N_CHUNKS           # 2560 genes per core (padded)
G_PAD = G_LOC * N_CORES        # 20480
BD = B * D                     # 1024
ROW = BD + S                   # 1152: [x row | w row] per gene
FREE = 512                     # max fp32 free dim per PSUM bank
N_FREE = BD // FREE            # 2
GROUP_SIZES = [4, 4, 3, 3, 3, 2, 1]
N_GROUPS = len(GROUP_SIZES)
N_WARM = 48                    # PE-ramp matmuls during first-DMA latency

MM_DT = mybir.dt.float16


def build_nc() -> bass.Bass:
    nc = Bacc("TRN2", target_bir_lowering=False)

    # xp[p, c*ROW + f] = packed row of gene (chunk c, partition p)
    xp_d = nc.declare_dram_parameter(
        "xp", [K, N_CHUNKS * ROW], MM_DT, isOutput=False
    )
    out = nc.declare_dram_parameter("out", [S, BD], MM_DT, isOutput=True)

    with TileContext(nc) as tc:
        with (
            tc.tile_pool(name="gp", bufs=1) as gp,
            tc.tile_pool(name="op", bufs=2) as op,
            tc.tile_pool(name="ps", bufs=1, space="PSUM") as ps,
        ):
            psums = [
                ps.tile([S, FREE], mybir.dt.float32, name=f"psum{j}")
                for j in range(N_FREE)
            ]
            warm_ps = ps.tile([K, K], mybir.dt.float32, name="warm_ps")
            warm = gp.tile([K, K], MM_DT, name="warm", tag="warm")
            nc.vector.memset(warm[:], 0.0)
            for _ in range(N_WARM):
                nc.tensor.matmul(
                    warm_ps[:], lhsT=warm[:], rhs=warm[:], start=True, stop=True
                )

            tiles = []
            start_chunk = []
            c0 = 0
            for g, sz in enumerate(GROUP_SIZES):
                g_t = gp.tile([K, sz * ROW], MM_DT, name=f"grp{g}", tag=f"grp{g}")
                nc.sync.dma_start(
                    out=g_t[:], in_=xp_d[:, c0 * ROW:(c0 + sz) * ROW]
                )
                tiles.append(g_t)
                start_chunk.append(c0)
                c0 += sz
            for g, sz in enumerate(GROUP_SIZES):
                for l in range(sz):
                    c = start_chunk[g] + l
                    base = l * ROW
                    for j in range(N_FREE):
                        nc.tensor.matmul(
                            psums[j][:],
                            lhsT=tiles[g][:, base + BD:base + ROW],
                            rhs=tiles[g][:, base + j * FREE:base + (j + 1) * FREE],
                            start=(c == 0),
                            stop=(c == N_CHUNKS - 1),
                        )
            for j in range(N_FREE):
                o_t = op.tile([S, FREE], MM_DT)
                if j % 2 == 0:
                    nc.vector.tensor_copy(out=o_t[:], in_=psums[j][:])
                else:
                    nc.scalar.copy(out=o_t[:], in_=psums[j][:])
                nc.sync.dma_start(out=out[:, j * FREE:(j + 1) * FREE], in_=o_t[:])
    nc.compile()
    return nc


_CACHE: dict = {}


def _get_nc() -> bass.Bass:
    if "nc" not in _CACHE:
        _CACHE["nc"] = build_nc()
    return _CACHE["nc"]


def _shard_inputs(x: np.ndarray, W: np.ndarray) -> list[dict[str, np.ndarray]]:
    # Packed per-gene rows [x[:, g, :].ravel() | W[:, g]] -> XW [G_PAD, ROW],
    # zero rows beyond G.  Then partition-major per core:
    # XP[i, p, c, :] = XW[i*G_LOC + c*K + p, :]
    XW = np.zeros((G_PAD, ROW), dtype=np.float16)
    XW[:G, :BD] = x.transpose(1, 0, 2).reshape(G, BD)
    XW[:G, BD:] = W.T
    XP = np.ascontiguousarray(
        XW.reshape(N_CORES, N_CHUNKS, K, ROW).transpose(0, 2, 1, 3)
    ).reshape(N_CORES, K, N_CHUNKS * ROW)
    return [{"xp": XP[i]} for i in range(N_CORES)]


def run(x: np.ndarray, W: np.ndarray, **spmd_kwargs):
    nc = _get_nc()
    in_maps = _shard_inputs(x, W)
    res = run_bass_kernel_spmd(nc, in_maps, list(range(N_CORES)), **spmd_kwargs)
    partial = np.zeros((S, BD), dtype=np.float32)
    for r in res.results:
        partial += r["out"].astype(np.float32)
    out = partial.reshape(S, B, D).transpose(1, 0, 2)
    return np.ascontiguousarray(out), res


def kernel(x: np.ndarray, W: np.ndarray) -> np.ndarray:
    out, _ = run(x, W)
    return out


# revision 5
# speedup vs baseline: 1.0853x; 1.0853x over previous
"""Trainium2 Bass kernel for nn_GeneSetPlaceholderAggregator.

Computes out[b,s,d] = sum_g x[b,g,d] * W[s,g]  (einsum 'bgd,sg->bsd')
with B=64, G=20000, D=16, S=128.

Strategy:
- Shard the contraction axis G across 8 cores (2500 genes each, zero-padded
  to 2560 = 20 chunks of 128).  Each core computes a full partial output
  [S=128, B*D=1024] via PSUM-accumulated matmuls (contraction on the
  partition dim); the host sums the 8 partials.
- fp16 operands: halves HBM traffic vs fp32 and runs the PE at 1 cycle/row
  with fast-weight-load (fp32r measured 1.5 cyc/row with serial LDWEIGHTS).
  Rounding error ~4e-4 rel, far under the 2e-2 gate.
- Warm-up matmuls on a zeroed tile during the first-DMA latency window trip
  the HAM activity ramp so real matmuls run at 2.4 GHz.
- Host packs per-gene rows [x | W] partition-major ([128, chunk, row]) so
  every DMA descriptor is a long contiguous run per partition.
- Output returned as fp16 [S, B*D] (host sums partials in fp32).
"""

import numpy as np

import concourse.mybir as mybir
from concourse import bass
from concourse.bacc import Bacc
from concourse.bass_utils import run_bass_kernel_spmd
from concourse.tile import TileContext

B, G, D, S = 64, 20000, 16, 128
N_CORES = 8
K = 128                        # contraction tile = partition dim
N_CHUNKS = 20                  # chunks per core
G_LOC = K * N_CHUNKS           # 2560 genes per core (padded)
G_PAD = G_LOC * N_CORES        # 20480
BD = B * D                     # 1024
ROW = BD + S                   # 1152: [x row | w row] per gene
FREE = 512                     # max fp32 free dim per PSUM bank
N_FREE = BD // FREE            # 2
GROUP_SIZES = [4, 4, 4, 3, 2, 2, 1]
N_GROUPS = len(GROUP_SIZES)
N_WARM = 24                    # PE-ramp matmuls during first-DMA latency

MM_DT = mybir.dt.float16


def build_nc() -> bass.Bass:
    nc = Bacc("TRN2", target_bir_lowering=False)

    # xp[p, c*ROW + f] = packed row of gene (chunk c, partition p)
    xp_d = nc.declare_dram_parameter(
        "xp", [K, N_CHUNKS * ROW], MM_DT, isOutput=False
    )
    out = nc.declare_dram_parameter("out", [S, BD], MM_DT, isOutput=True)

    with TileContext(nc) as tc:
        with (
            tc.tile_pool(name="gp", bufs=1) as gp,
            tc.tile_pool(name="op", bufs=2) as op,
            tc.tile_pool(name="ps", bufs=1, space="PSUM") as ps,
        ):
            psums = [
                ps.tile([S, FREE], mybir.dt.float32, name=f"psum{j}")
                for j in range(N_FREE)
            ]
            warm_ps = ps.tile([K, K], mybir.dt.float32, name="warm_ps")
            warm = gp.tile([K, K], MM_DT, name="warm", tag="warm")
            nc.vector.memset(warm[:], 0.0)
            for _ in range(N_WARM):
                nc.tensor.matmul(
                    warm_ps[:], lhsT=warm[:], rhs=warm[:], start=True, stop=True
                )

            tiles = []
            start_chunk = []
            c0 = 0
            for g, sz in enumerate(GROUP_SIZES):
                g_t = gp.tile([K, sz * ROW], MM_DT, name=f"grp{g}", tag=f"grp{g}")
                nc.sync.dma_start(
                    out=g_t[:], in_=xp_d[:, c0 * ROW:(c0 + sz) * ROW]
                )
                tiles.append(g_t)
                start_chunk.append(c0)
                c0 += sz
            for g, sz in enumerate(GROUP_SIZES):
                for l in range(sz):
                    c = start_chunk[g] + l
                    base = l * ROW
                    for j in range(N_FREE):
                        nc.tensor.matmul(
                            psums[j][:],
                            lhsT=tiles[g][:, base + BD:base + ROW],
                            rhs=tiles[g][:, base + j * FREE:base + (j + 1) * FREE],
                            start=(c == 0),
                            stop=(c == N_CHUNKS - 1),
                        )
            for j in range(N_FREE):
                o_t = op.tile([S, FREE], MM_DT)
                if j % 2 == 0:
                    nc.vector.tensor_copy(out=o_t[:], in_=psums[j][:])
                else:
                    nc.scalar.copy(out=o_t[:], in_=psums[j][:])
                nc.sync.dma_start(out=out[:, j * FREE:(j + 1) * FREE], in_=o_t[:])
    nc.compile()
    return nc


_CACHE: dict = {}


def _get_nc() -> bass.Bass:
    if "nc" not in _CACHE:
        _CACHE["nc"] = build_nc()
    return _CACHE["nc"]


def _shard_inputs(x: np.ndarray, W: np.ndarray) -> list[dict[str, np.ndarray]]:
    # Packed per-gene rows [x[:, g, :].ravel() | W[:, g]] -> XW [G_PAD, ROW],
    # zero rows beyond G.  Then partition-major per core:
    # XP[i, p, c, :] = XW[i*G_LOC + c*K + p, :]
    XW = np.zeros((G_PAD, ROW), dtype=np.float16)
    XW[:G, :BD] = x.transpose(1, 0, 2).reshape(G, BD)
    XW[:G, BD:] = W.T
    XP = np.ascontiguousarray(
        XW.reshape(N_CORES, N_CHUNKS, K, ROW).transpose(0, 2, 1, 3)
    ).reshape(N_CORES, K, N_CHUNKS * ROW)
    return [{"xp": XP[i]} for i in range(N_CORES)]


def run(x: np.ndarray, W: np.ndarray, **spmd_kwargs):
    nc = _get_nc()
    in_maps = _shard_inputs(x, W)
    res = run_bass_kernel_spmd(nc, in_maps, list(range(N_CORES)), **spmd_kwargs)
    partial = np.zeros((S, BD), dtype=np.float32)
    for r in res.results:
        partial += r["out"].astype(np.float32)
    out = partial.reshape(S, B, D).transpose(1, 0, 2)
    return np.ascontiguousarray(out), res


def kernel(x: np.ndarray, W: np.ndarray) -> np.ndarray:
    out, _ = run(x, W)
    return out
